# revision 39
# baseline (speedup 1.0000x reference)
"""BiLSTM layer kernel for 8 Trainium2 NeuronCores.

Problem: B=32, T=256, D=1024, H=1024 bidirectional LSTM.
Sharding: data-parallel over batch (4 rows/core) x 2 directions = 8
sequences per core; weights replicated; time recurrence local per core.

Per-core algorithm:
  Phase 1: pre-gates PRE[s,t,:] = x[s,t,:] @ Wx + b (bf16, dense m=128
           matmuls) stored to a DRAM scratch tensor as bf16 hi+lo. The
           first 32 fwd/bwd steps' worth runs upfront; the rest is
           emitted one (m,n)-unit per recurrence step as PE filler that
           keeps the HAM clock-gate warm through the elementwise tails.
  Phase 2: 256 recurrent steps, fwd and bwd interleaved (8 sequences =
           4 batch x 2 directions share each Wh stream). Per step the
           gates accumulate into three psum tiles ([f,i], [g], [o]),
           round order fi -> g -> o, so tanh(f,i)+A overlap the g
           rounds and the g-chain (Bv, S, tc) overlaps the o rounds.
           h^T is the stationary operand (8 cols x 4 tile_position
           column strips, concurrent streams); the elementwise cell
           update runs full-width (all 4 bands at once, garbage lanes
           confined); h returns to stationary layout via PE transposes
           in band-major pairs (kk-major singles crash the device),
           band 0 first so next step's k=0 round starts early. The
           next step's pre-gate psum preload (sel rounds) is emitted
           before this step's transposes to avoid head-of-line
           blocking in the PE queue.

All activations are computed as tanh:  sigmoid(x) = (tanh(x/2)+1)/2.
The x/2 scaling for f,i,o gate columns is baked into the weights on the
host; the (t+1)/2 is folded into the elementwise ops, with h kept as 2*h
internally (Wh rows are pre-halved to compensate; the host halves the
final output).

Gate column order is (f, i, o, c~), grouped by h-quarter band:
  col = g*1024 + gate*256 + n   for h index  g*256 + n.
"""

import numpy as np

B, T, D, H = 32, 256, 1024, 1024
NCORES = 8
BS = B // NCORES           # batch rows per core
KT = D // 128              # k-tiles over the hidden/contraction dim
G4 = 4 * H                 # gate columns
RING = 8                   # output ring depth (steps per output DMA)

_BUILT = None


def _build(nc_T=T):
    import concourse.bass as bass
    import concourse.bacc as bacc
    import concourse.tile as tile
    from concourse import mybir

    f32 = mybir.dt.float32
    f32r = mybir.dt.float32r
    bf16 = mybir.dt.bfloat16
    Tanh = mybir.ActivationFunctionType.Tanh
    Sigm = mybir.ActivationFunctionType.Sigmoid
    MUL = mybir.AluOpType.mult
    ADD = mybir.AluOpType.add

    nrows = BS * nc_T      # rows of the phase-1 matmul

    nc = bacc.Bacc("TRN2", target_bir_lowering=False)

    xT = nc.dram_tensor("xT", [D + 1, nrows], bf16, kind="ExternalInput")
    wxb = nc.dram_tensor("wxb", [D + 1, G4], bf16, kind="ExternalInput")
    wh = nc.dram_tensor("wh", [D, G4], bf16, kind="ExternalInput")
    sel = nc.dram_tensor("sel", [64, 32], bf16, kind="ExternalInput")
    id8 = nc.dram_tensor("id8", [128, 8], f32, kind="ExternalInput")
    outd = nc.dram_tensor("out", [nc_T, 2, 4, BS, 256], f32,
                          kind="ExternalOutput")
    import os
    norec = bool(os.environ.get("K_NOREC"))
    nob = bool(os.environ.get("K_NOB"))
    notrans = bool(os.environ.get("K_NOTRANS"))
    pred = nc.dram_tensor("pre", [nc_T, 2, 4 * BS, 1024], bf16,
                          kind=("ExternalOutput"
                                if os.environ.get("K_P1DBG") else "Internal"))

    with tile.TileContext(nc) as tc:
        _emit(nc, tc, bass, mybir, nc_T,
              xT, wxb, wh, sel, id8, outd, pred,
              f32, f32r, bf16, Tanh, Sigm, MUL, ADD, norec, notrans,
              nob)
    nc.finalize()
    return nc


def _emit(nc, tc, bass, mybir, nc_T, xT, wxb, wh, sel, id8, outd, pred,
          f32, f32r, bf16, Tanh, Sigm, MUL, ADD,
          norec=False, notrans=False, nob=False):
    from contextlib import ExitStack

    nrows = BS * nc_T
    act = nc.scalar
    dve = nc.vector
    pool = nc.gpsimd
    pe = nc.tensor
    sdma = nc.sync

    with ExitStack() as ctx:
        # ------------- constants / big weights -------------
        singles = ctx.enter_context(tc.tile_pool(name="singles", bufs=1))
        sel_sb = singles.tile([64, 32], bf16)
        id8_sb = singles.tile([128, 8], f32)
        sdma.dma_start(out=sel_sb, in_=sel[:, :])
        sdma.dma_start(out=id8_sb, in_=id8[:, :])

        # ------------- persistent weights / inputs -------------
        wpool = ctx.enter_context(tc.tile_pool(name="wpool", bufs=1))
        wh_sb = wpool.tile([128, KT, G4], bf16, tag="wh")
        sdma.dma_start(
            out=wh_sb, in_=wh[:, :].rearrange("(k p) c -> p k c", p=128))
        wxb_sb = wpool.tile([128, KT, G4], bf16, tag="wx")
        sdma.dma_start(
            out=wxb_sb,
            in_=wxb[0:D, :].rearrange("(k p) c -> p k c", p=128))
        wxbrow = wpool.tile([1, G4], bf16, tag="wxr")
        sdma.dma_start(out=wxbrow, in_=wxb[D:D + 1, :])
        xT_sb = wpool.tile([128, KT, nrows], bf16, tag="xT")
        sdma.dma_start(
            out=xT_sb,
            in_=xT[0:D, :].rearrange("(k p) c -> p k c", p=128))
        ones_sb = wpool.tile([1, nrows], bf16, tag="ones")
        sdma.dma_start(out=ones_sb, in_=xT[D:D + 1, :])

        # ---------------- phase 1: pre-gates (x @ Wx + b) ----------------
        # t-major rows: row r = t * BS + s covers t-tile m = 32 t-steps.
        # m-tile order pairs (0,7),(1,6),... so the first pair covers the
        # first 32 fwd and bwd steps; one (m,n) unit is emitted per
        # recurrence step after the first 16 upfront units, keeping the
        # PE warm through the elementwise tails.
        p1out = ctx.enter_context(tc.tile_pool(name="p1out", bufs=3))
        p1ps = ctx.enter_context(tc.tile_pool(name="p1ps", bufs=1,
                                              space="PSUM"))

        n_mtiles = (nrows + 127) // 128
        morder = []
        for j in range((n_mtiles + 1) // 2):
            morder.append(j)
            if n_mtiles - 1 - j != j:
                morder.append(n_mtiles - 1 - j)

        p1_state = {}

        def emit_p1_part(m, n, part):
            """part 0: first 5 k-matmuls; part 1: rest + copies + DMA."""
            if part == 0:
                p1_state["pt"] = p1ps.tile([128, 512], f32, name="pt")
            pt = p1_state["pt"]
            krange = range(0, 5) if part == 0 else range(5, KT)
            for k in krange:
                pe.matmul(
                    pt,
                    lhsT=xT_sb[:, k, m * 128:(m + 1) * 128],
                    rhs=wxb_sb[:, k, n * 512:(n + 1) * 512],
                    start=(k == 0), stop=False,
                )
            if part == 0:
                return
            pe.matmul(
                pt,
                lhsT=ones_sb[:, m * 128:(m + 1) * 128],
                rhs=wxbrow[:, n * 512:(n + 1) * 512],
                start=False, stop=True,
            )
            ot = p1out.tile([128, 512], bf16, name="ot")
            ol = p1out.tile([128, 512], bf16, name="ol")
            dve.tensor_copy(ot, pt)
            dve.tensor_sub(ol, pt, ot)
            # rows r = m*128 + p -> (s = p // 32, t = 32*m + p % 32)
            g = n // 2
            half = n % 2
            t0 = 32 * m
            for s in range(BS):
                sdma.dma_start(
                    out=pred[t0:t0 + 32, 0, 4 * g + s,
                             half * 512:(half + 1) * 512],
                    in_=ot[s * 32:(s + 1) * 32, :],
                )
                sdma.dma_start(
                    out=pred[t0:t0 + 32, 1, 4 * g + s,
                             half * 512:(half + 1) * 512],
                    in_=ol[s * 32:(s + 1) * 32, :],
                )

        def emit_p1_dummy():
            """PE warm-keeper for steps past the phase-1 work: a few
            full-array matmuls into the (dead) p1 psum tile."""
            pt = p1ps.tile([128, 512], f32, name="pt")
            for k in range(4):
                pe.matmul(
                    pt,
                    lhsT=xT_sb[:, k, 0:128],
                    rhs=wh_sb[:, k, 0:512],
                    start=(k == 0), stop=(k == 3),
                )

        # all units upfront: measured per-step filler made steps ~1.6us
        # slower than bare ones, so dense upfront (~80us) is cheaper
        p1_units = [(m, n) for m in morder for n in range(8)]
        for m, n in p1_units:
            emit_p1_part(m, n, 0)
            emit_p1_part(m, n, 1)

        def emit_p1_step(tau):
            pass

        # ---------------- phase 2: recurrence ----------------

        state = ctx.enter_context(tc.tile_pool(name="state", bufs=1))
        prepool = ctx.enter_context(tc.tile_pool(name="pre", bufs=2))
        ewpool = ctx.enter_context(tc.tile_pool(name="ew", bufs=2))
        ringpool = ctx.enter_context(tc.tile_pool(name="ring", bufs=1))
        gpool = ctx.enter_context(tc.tile_pool(name="gp", bufs=2,
                                               space="PSUM"))
        tpool = ctx.enter_context(tc.tile_pool(name="tp", bufs=1,
                                               space="PSUM"))

        # h^T stationary: one tile, band b k-subtile kk at cols b*16+kk*8
        hT = state.tile([128, 64], bf16, tag="hT", name="hT")
        # cell state as S = 2*c, per band rows, [8, 256] in band rows
        S_sb = state.tile([128, 256], f32, tag="S")
        dve.memset(S_sb, 0.0)

        ring_f = ringpool.tile([128, RING, 256], f32, tag="rf")
        ring_b = ringpool.tile([128, RING, 256], f32, tag="rb")

        def emit_sel(tau):
            """DMA pre-gates for step tau and preload the psum banks."""
            pfb = prepool.tile([64, 1024], bf16, tag="pfb")
            sdma.dma_start(out=pfb[0:32, :], in_=pred[tau, :, :, :])
            sdma.dma_start(out=pfb[32:64, :],
                           in_=pred[nc_T - 1 - tau, :, :, :])
            gp_fi = gpool.tile([128, 512], f32, tag="gfi", name="gfi")
            gp_g = gpool.tile([128, 256], f32, tag="gg", name="gg")
            gp_o = gpool.tile([128, 256], f32, tag="go", name="go")
            banks = ((gp_fi, 0, 512), (gp_g, 512, 256), (gp_o, 768, 256))
            for gt, c0, cw in banks:
                for b in range(4):
                    pe.matmul(
                        gt[32 * b:32 * b + 8, :],
                        lhsT=sel_sb[:, b * 8:(b + 1) * 8],
                        rhs=pfb[:, c0:c0 + cw],
                        start=True, stop=(tau == 0),
                        tile_position=(0, 32 * b),
                        skip_group_check=True,
                    )
            return banks

        # k-rounds in natural order: band-major transposes produce the
        # hT pieces in exactly this consumption order
        korder = list(range(KT))
        import os as _os
        pipelined = not bool(_os.environ.get("K_NOPIPE"))
        banks_cur = emit_sel(0) if pipelined else None

        for tau in range(nc_T):
            slot = tau % RING
            bslot = RING - 1 - slot
            if not pipelined:
                banks_cur = emit_sel(tau)

            # --- recurrent k-rounds, grouped fi -> g -> o ---
            for gt, c0, cw in banks_cur:
                for k in (korder if (tau > 0 and not norec) else []):
                    kb = k // 2  # band whose h feeds this k-tile
                    for b in range(4):
                        pe.matmul(
                            gt[32 * b:32 * b + 8, :],
                            lhsT=hT[:, kb * 16 + (k % 2) * 8:
                                    kb * 16 + (k % 2) * 8 + 8],
                            rhs=wh_sb[:, k, b * 1024 + c0:
                                      b * 1024 + c0 + cw],
                            start=False, stop=(k == KT - 1),
                            tile_position=(0, 32 * b),
                            skip_group_check=True,
                        )
            gp_fi, gp_g, gp_o = (t for t, _, _ in banks_cur)
            # next step's pre-gate rounds fill the PE during this step's
            # elementwise tail (independent of h)
            banks_next = (emit_sel(tau + 1)
                          if pipelined and tau + 1 < nc_T else None)
            # phase-1 work (or a dummy warm-keeper) as PE filler during
            # the elementwise tail: keeps the HAM clock-gate at 8/8
            emit_p1_step(tau)

            # --- elementwise phase, full width (all 4 bands at once;
            # non-band partitions compute garbage that stays confined) ---
            tg = ewpool.tile([128, 768], f32, tag="tg")
            so = ewpool.tile([128, 256], f32, tag="so")
            tc_t = ewpool.tile([128, 256], f32, tag="tc")
            a_t = ewpool.tile([128, 256], f32, tag="a")
            b_t = ewpool.tile([128, 256], f32, tag="b")
            # cols: [f | i | g~ | o]; f,i pre-scaled by 1/2 in W
            act.activation(tg[:, 0:512], gp_fi, Tanh)
            # A = (tf + 1) * S_prev      (S = 2*c)
            dve.scalar_tensor_tensor(
                a_t, tg[:, 0:256], 1.0, S_sb, op0=ADD, op1=MUL)
            act.activation(tg[:, 512:768], gp_g, Tanh)
            act.activation(so, gp_o, Sigm)
            # post-fi chain in 128-col chunks so h's first half (and the
            # kk=0 transpose) complete as early as possible
            for ch in range(2):
                sl = slice(ch * 128, (ch + 1) * 128)
                # Bv = (ti + 1) * tg~
                dve.scalar_tensor_tensor(
                    b_t[:, sl], tg[:, 256 + ch * 128:384 + ch * 128], 1.0,
                    tg[:, 512 + ch * 128:640 + ch * 128], op0=ADD, op1=MUL)
                # S_new = A * 0.5 + Bv   (= 2*c_new)
                dve.scalar_tensor_tensor(
                    S_sb[:, sl], a_t[:, sl], 0.5, b_t[:, sl],
                    op0=MUL, op1=ADD)
                # tc = tanh(c_new) = tanh(S_new / 2)
                act.activation(tc_t[:, sl], S_sb[:, sl], Tanh, scale=0.5)
                # h = sigmoid(o) * tanh(c)
                dve.tensor_mul(ring_f[:, slot, sl], so[:, sl], tc_t[:, sl])

                # --- transposes (band-major pairs only: alternating
                # row-group singles crash the device) ---
                if notrans or norec:
                    continue
                if ch == 0:
                    # band 0 kk=0 right after h's first half: unblocks
                    # next step's k=0 round
                    tp = tpool.tile([128, 64], f32, tag="tp", name="tp")
                    pe.transpose(
                        tp[:, 0:8],
                        in_=ring_f[0:8, slot, 0:128],
                        identity=id8_sb[0:8, :],
                        tile_position=(0, 0),
                    )
                    dve.tensor_copy(hT[:, 0:8], tp[:, 0:8])
                else:
                    pe.transpose(
                        tp[:, 8:16],
                        in_=ring_f[0:8, slot, 128:256],
                        identity=id8_sb[0:8, :],
                        tile_position=(0, 0),
                    )
                    dve.tensor_copy(hT[:, 8:16], tp[:, 8:16])
                    for b in range(1, 4):
                        rows = slice(32 * b, 32 * b + 8)
                        for kk in range(2):
                            pe.transpose(
                                tp[:, b * 16 + kk * 8:b * 16 + kk * 8 + 8],
                                in_=ring_f[rows, slot,
                                           kk * 128:(kk + 1) * 128],
                                identity=id8_sb[rows, :],
                                tile_position=(32 * b, 0),
                            )
                        dve.tensor_copy(hT[:, b * 16:b * 16 + 16],
                                        tp[:, b * 16:b * 16 + 16])

            if not nob:
                dve.tensor_mul(ring_b[:, bslot, :], so, tc_t)
            if pipelined:
                banks_cur = banks_next

            # --- output DMA every RING steps (partial at sequence end) ---
            if slot == RING - 1 or tau == nc_T - 1:
                cnt = slot + 1
                t0 = tau - slot
                bt0 = nc_T - 1 - tau
                for b in range(4):
                    sdma.dma_start(
                        out=outd[t0:t0 + cnt, 0, b, :, :]
                        .rearrange("t s n -> s t n"),
                        in_=ring_f[32 * b:32 * b + 4, 0:cnt, :],
                    )
                    sdma.dma_start(
                        out=outd[bt0:bt0 + cnt, 1, b, :, :]
                        .rearrange("t s n -> s t n"),
                        in_=ring_b[32 * b + 4:32 * b + 8,
                                   RING - cnt:RING, :],
                    )


def _prep_inputs(x, Wf, bf, Wi, bi, Wc, bc, Wo, bo, nc_T=T):
    """Host-side input preparation -> list of per-core in_maps."""
    # gate order (f, i, o, c~); f/i/o columns scaled by 1/2 (sigmoid via
    # tanh); Wh rows halved to absorb h being carried as 2*h.
    W = np.stack([Wf, Wi, Wc, Wo], axis=1)          # (2048, 4, 1024)
    bv = np.stack([bf, bi, bc, bo], axis=0)         # (4, 1024)
    scale = np.array([0.5, 0.5, 1.0, 1.0], dtype=np.float32)
    W = W * scale[None, :, None]
    bv = bv * scale[:, None]
    # column remap: col' = g*1024 + gate*256 + n for H index g*256+n
    W4 = np.ascontiguousarray(
        W.reshape(2048, 4, 4, 256).transpose(0, 2, 1, 3).reshape(2048, G4))
    b4 = np.ascontiguousarray(
        bv.reshape(4, 4, 256).transpose(1, 0, 2).reshape(G4))
    import ml_dtypes
    Wh = np.ascontiguousarray(W4[:H].astype(ml_dtypes.bfloat16))
    Wx = W4[H:]
    wxb = np.ascontiguousarray(
        np.concatenate([Wx, b4[None, :]], axis=0)
        .astype(ml_dtypes.bfloat16))  # (1025, 4096)
    # selection matrix rows (dir, hi/lo, g, bs); sums hi+lo parts
    selm = np.zeros((64, 32), dtype=np.float32)
    for b_ in range(4):
        for sp in range(8):
            for part in range(2):
                selm[(sp // 4) * 32 + part * 16 + b_ * 4 + (sp % 4),
                     b_ * 8 + sp] = 1.0
    selm = selm.astype(ml_dtypes.bfloat16)
    id8 = np.ascontiguousarray(np.tile(np.eye(8, dtype=np.float32), (16, 1)))

    in_maps = []
    for c in range(NCORES):
        # rows blocked by 32-step t-tile, s-major within the tile:
        # r = (t // 32) * 128 + s * 32 + t % 32
        xc = np.ascontiguousarray(
            x[BS * c:BS * c + BS, :nc_T, :]
            .reshape(BS, nc_T // 32, 32, D).transpose(1, 0, 2, 3)
        ).reshape(nc_T * BS, D)
        xTc = np.concatenate(
            [xc.T, np.ones((1, BS * nc_T), dtype=np.float32)], axis=0)
        in_maps.append({
            "xT": np.ascontiguousarray(xTc.astype(ml_dtypes.bfloat16)),
            "wxb": wxb,
            "wh": Wh,
            "sel": selm,
            "id8": id8,
        })
    return in_maps


def _assemble(results, nc_T=T):
    """results: list of dicts with 'out' (T, 2, 4, BS, 256) = 2*h."""
    full = np.empty((B, nc_T, 2 * H), dtype=np.float32)
    for c in range(NCORES):
        o = results[c]["out"]                       # (T, 2, 4, BS, 256)
        o2 = o.transpose(3, 0, 1, 2, 4).reshape(BS, nc_T, 2 * H)
        full[BS * c:BS * c + BS] = o2
    return full


def kernel(**inputs):
    global _BUILT
    from concourse.bass_utils import run_bass_kernel_spmd

    x = np.asarray(inputs["x"], dtype=np.float32)
    args = [np.asarray(inputs[k], dtype=np.float32)
            for k in ("Wf", "bf", "Wi", "bi", "Wc", "bc", "Wo", "bo")]
    in_maps = _prep_inputs(x, args[0], args[1], args[2], args[3],
                           args[4], args[5], args[6], args[7])
    if _BUILT is None:
        _BUILT = _build()
    res = run_bass_kernel_spmd(_BUILT, in_maps, core_ids=list(range(NCORES)))
    return _assemble(res.results)



# revision 40
# speedup vs baseline: 1.0530x; 1.0530x over previous
"""BiLSTM layer kernel for 8 Trainium2 NeuronCores.

Problem: B=32, T=256, D=1024, H=1024 bidirectional LSTM.
Sharding: data-parallel over batch (4 rows/core) x 2 directions = 8
sequences per core; weights replicated; time recurrence local per core.

Per-core algorithm:
  Phase 1: pre-gates PRE[s,t,:] = x[s,t,:] @ Wx + b (bf16, dense m=128
           matmuls) stored to a DRAM scratch tensor as bf16 hi+lo. The
           first 32 fwd/bwd steps' worth runs upfront; the rest is
           emitted one (m,n)-unit per recurrence step as PE filler that
           keeps the HAM clock-gate warm through the elementwise tails.
  Phase 2: 256 recurrent steps, fwd and bwd interleaved (8 sequences =
           4 batch x 2 directions share each Wh stream). Per step the
           gates accumulate into three psum tiles ([f,i], [g], [o]),
           round order fi -> g -> o, so tanh(f,i)+A overlap the g
           rounds and the g-chain (Bv, S, tc) overlaps the o rounds.
           h^T is the stationary operand (8 cols x 4 tile_position
           column strips, concurrent streams); the elementwise cell
           update runs full-width (all 4 bands at once, garbage lanes
           confined); h returns to stationary layout via PE transposes
           in band-major pairs (kk-major singles crash the device),
           band 0 first so next step's k=0 round starts early. The
           next step's pre-gate psum preload (sel rounds) is emitted
           before this step's transposes to avoid head-of-line
           blocking in the PE queue.

All activations are computed as tanh:  sigmoid(x) = (tanh(x/2)+1)/2.
The x/2 scaling for f,i,o gate columns is baked into the weights on the
host; the (t+1)/2 is folded into the elementwise ops, with h kept as 2*h
internally (Wh rows are pre-halved to compensate; the host halves the
final output).

Gate column order is (f, i, o, c~), grouped by h-quarter band:
  col = g*1024 + gate*256 + n   for h index  g*256 + n.
"""

import numpy as np

B, T, D, H = 32, 256, 1024, 1024
NCORES = 8
BS = B // NCORES           # batch rows per core
KT = D // 128              # k-tiles over the hidden/contraction dim
G4 = 4 * H                 # gate columns
RING = 8                   # output ring depth (steps per output DMA)

_BUILT = None


def _build(nc_T=T):
    import concourse.bass as bass
    import concourse.bacc as bacc
    import concourse.tile as tile
    from concourse import mybir

    f32 = mybir.dt.float32
    f32r = mybir.dt.float32r
    bf16 = mybir.dt.bfloat16
    Tanh = mybir.ActivationFunctionType.Tanh
    Sigm = mybir.ActivationFunctionType.Sigmoid
    MUL = mybir.AluOpType.mult
    ADD = mybir.AluOpType.add

    nrows = BS * nc_T      # rows of the phase-1 matmul

    nc = bacc.Bacc("TRN2", target_bir_lowering=False)

    xT = nc.dram_tensor("xT", [D + 1, nrows], bf16, kind="ExternalInput")
    wxb = nc.dram_tensor("wxb", [D + 1, G4], bf16, kind="ExternalInput")
    wh = nc.dram_tensor("wh", [D, G4], bf16, kind="ExternalInput")
    sel = nc.dram_tensor("sel", [64, 32], bf16, kind="ExternalInput")
    id8 = nc.dram_tensor("id8", [128, 8], f32, kind="ExternalInput")
    outd = nc.dram_tensor("out", [nc_T, 2, 4, BS, 256], f32,
                          kind="ExternalOutput")
    import os
    norec = bool(os.environ.get("K_NOREC"))
    nob = bool(os.environ.get("K_NOB"))
    notrans = bool(os.environ.get("K_NOTRANS"))
    pred = nc.dram_tensor("pre", [nc_T, 2, 4 * BS, 1024], bf16,
                          kind=("ExternalOutput"
                                if os.environ.get("K_P1DBG") else "Internal"))

    with tile.TileContext(nc) as tc:
        _emit(nc, tc, bass, mybir, nc_T,
              xT, wxb, wh, sel, id8, outd, pred,
              f32, f32r, bf16, Tanh, Sigm, MUL, ADD, norec, notrans,
              nob)
    nc.finalize()
    return nc


def _emit(nc, tc, bass, mybir, nc_T, xT, wxb, wh, sel, id8, outd, pred,
          f32, f32r, bf16, Tanh, Sigm, MUL, ADD,
          norec=False, notrans=False, nob=False):
    from contextlib import ExitStack

    nrows = BS * nc_T
    act = nc.scalar
    dve = nc.vector
    pool = nc.gpsimd
    pe = nc.tensor
    sdma = nc.sync

    with ExitStack() as ctx:
        # ------------- constants / big weights -------------
        singles = ctx.enter_context(tc.tile_pool(name="singles", bufs=1))
        sel_sb = singles.tile([64, 32], bf16)
        id8_sb = singles.tile([128, 8], f32)
        sdma.dma_start(out=sel_sb, in_=sel[:, :])
        sdma.dma_start(out=id8_sb, in_=id8[:, :])

        # ------------- persistent weights / inputs -------------
        wpool = ctx.enter_context(tc.tile_pool(name="wpool", bufs=1))
        wh_sb = wpool.tile([128, KT, G4], bf16, tag="wh")
        sdma.dma_start(
            out=wh_sb, in_=wh[:, :].rearrange("(k p) c -> p k c", p=128))
        wxb_sb = wpool.tile([128, KT, G4], bf16, tag="wx")
        sdma.dma_start(
            out=wxb_sb,
            in_=wxb[0:D, :].rearrange("(k p) c -> p k c", p=128))
        wxbrow = wpool.tile([1, G4], bf16, tag="wxr")
        sdma.dma_start(out=wxbrow, in_=wxb[D:D + 1, :])
        xT_sb = wpool.tile([128, KT, nrows], bf16, tag="xT")
        sdma.dma_start(
            out=xT_sb,
            in_=xT[0:D, :].rearrange("(k p) c -> p k c", p=128))
        ones_sb = wpool.tile([1, nrows], bf16, tag="ones")
        sdma.dma_start(out=ones_sb, in_=xT[D:D + 1, :])

        # ---------------- phase 1: pre-gates (x @ Wx + b) ----------------
        # t-major rows: row r = t * BS + s covers t-tile m = 32 t-steps.
        # m-tile order pairs (0,7),(1,6),... so the first pair covers the
        # first 32 fwd and bwd steps; one (m,n) unit is emitted per
        # recurrence step after the first 16 upfront units, keeping the
        # PE warm through the elementwise tails.
        p1out = ctx.enter_context(tc.tile_pool(name="p1out", bufs=3))
        p1ps = ctx.enter_context(tc.tile_pool(name="p1ps", bufs=1,
                                              space="PSUM"))

        n_mtiles = (nrows + 127) // 128
        morder = []
        for j in range((n_mtiles + 1) // 2):
            morder.append(j)
            if n_mtiles - 1 - j != j:
                morder.append(n_mtiles - 1 - j)

        p1_state = {}

        def emit_p1_part(m, n, part):
            """part 0: first 5 k-matmuls; part 1: rest + copies + DMA."""
            if part == 0:
                p1_state["pt"] = p1ps.tile([128, 512], f32, name="pt")
            pt = p1_state["pt"]
            krange = range(0, 5) if part == 0 else range(5, KT)
            for k in krange:
                pe.matmul(
                    pt,
                    lhsT=xT_sb[:, k, m * 128:(m + 1) * 128],
                    rhs=wxb_sb[:, k, n * 512:(n + 1) * 512],
                    start=(k == 0), stop=False,
                )
            if part == 0:
                return
            pe.matmul(
                pt,
                lhsT=ones_sb[:, m * 128:(m + 1) * 128],
                rhs=wxbrow[:, n * 512:(n + 1) * 512],
                start=False, stop=True,
            )
            ot = p1out.tile([128, 512], bf16, name="ot")
            ol = p1out.tile([128, 512], bf16, name="ol")
            dve.tensor_copy(ot, pt)
            dve.tensor_sub(ol, pt, ot)
            # rows r = m*128 + p -> (s = p // 32, t = 32*m + p % 32)
            g = n // 2
            half = n % 2
            t0 = 32 * m
            for s in range(BS):
                sdma.dma_start(
                    out=pred[t0:t0 + 32, 0, 4 * g + s,
                             half * 512:(half + 1) * 512],
                    in_=ot[s * 32:(s + 1) * 32, :],
                )
                sdma.dma_start(
                    out=pred[t0:t0 + 32, 1, 4 * g + s,
                             half * 512:(half + 1) * 512],
                    in_=ol[s * 32:(s + 1) * 32, :],
                )

        def emit_p1_dummy():
            """PE warm-keeper for steps past the phase-1 work: a few
            full-array matmuls into the (dead) p1 psum tile."""
            pt = p1ps.tile([128, 512], f32, name="pt")
            for k in range(4):
                pe.matmul(
                    pt,
                    lhsT=xT_sb[:, k, 0:128],
                    rhs=wh_sb[:, k, 0:512],
                    start=(k == 0), stop=(k == 3),
                )

        p1_units = [(m, n) for m in morder for n in range(8)]
        for m, n in p1_units[:16]:
            emit_p1_part(m, n, 0)
            emit_p1_part(m, n, 1)
        # units 16..47 one per step (steps 0..31), units 48..63 as
        # half-units (steps 32..63) -- keeps >=32 steps of slack between
        # a pred write and its first reader
        p1_sched = [(m, n, None) for m, n in p1_units[16:48]]
        p1_sched += [(m, n, p) for m, n in p1_units[48:] for p in (0, 1)]

        def emit_p1_step(tau):
            if tau < len(p1_sched):
                m, n, p = p1_sched[tau]
                if p is None:
                    emit_p1_part(m, n, 0)
                    emit_p1_part(m, n, 1)
                else:
                    emit_p1_part(m, n, p)

        # ---------------- phase 2: recurrence ----------------

        state = ctx.enter_context(tc.tile_pool(name="state", bufs=1))
        prepool = ctx.enter_context(tc.tile_pool(name="pre", bufs=2))
        ewpool = ctx.enter_context(tc.tile_pool(name="ew", bufs=2))
        ringpool = ctx.enter_context(tc.tile_pool(name="ring", bufs=1))
        gpool = ctx.enter_context(tc.tile_pool(name="gp", bufs=2,
                                               space="PSUM"))
        tpool = ctx.enter_context(tc.tile_pool(name="tp", bufs=1,
                                               space="PSUM"))

        # h^T stationary: one tile, band b k-subtile kk at cols b*16+kk*8
        hT = state.tile([128, 64], bf16, tag="hT", name="hT")
        # cell state as S = 2*c, per band rows, [8, 256] in band rows
        S_sb = state.tile([128, 256], f32, tag="S")
        dve.memset(S_sb, 0.0)

        ring_f = ringpool.tile([128, RING, 256], f32, tag="rf")
        ring_b = ringpool.tile([128, RING, 256], f32, tag="rb")

        def emit_sel(tau):
            """DMA pre-gates for step tau and preload the psum banks."""
            pfb = prepool.tile([64, 1024], bf16, tag="pfb")
            sdma.dma_start(out=pfb[0:32, :], in_=pred[tau, :, :, :])
            sdma.dma_start(out=pfb[32:64, :],
                           in_=pred[nc_T - 1 - tau, :, :, :])
            gp_fi = gpool.tile([128, 512], f32, tag="gfi", name="gfi")
            gp_g = gpool.tile([128, 256], f32, tag="gg", name="gg")
            gp_o = gpool.tile([128, 256], f32, tag="go", name="go")
            banks = ((gp_fi, 0, 512), (gp_g, 512, 256), (gp_o, 768, 256))
            for gt, c0, cw in banks:
                for b in range(4):
                    pe.matmul(
                        gt[32 * b:32 * b + 8, :],
                        lhsT=sel_sb[:, b * 8:(b + 1) * 8],
                        rhs=pfb[:, c0:c0 + cw],
                        start=True, stop=(tau == 0),
                        tile_position=(0, 32 * b),
                        skip_group_check=True,
                    )
            return banks

        # k-rounds in natural order: band-major transposes produce the
        # hT pieces in exactly this consumption order
        korder = list(range(KT))
        import os as _os
        pipelined = not bool(_os.environ.get("K_NOPIPE"))
        banks_cur = emit_sel(0) if pipelined else None

        for tau in range(nc_T):
            slot = tau % RING
            bslot = RING - 1 - slot
            if not pipelined:
                banks_cur = emit_sel(tau)

            # --- recurrent k-rounds, grouped fi -> g -> o ---
            for gt, c0, cw in banks_cur:
                for k in (korder if (tau > 0 and not norec) else []):
                    kb = k // 2  # band whose h feeds this k-tile
                    for b in range(4):
                        pe.matmul(
                            gt[32 * b:32 * b + 8, :],
                            lhsT=hT[:, kb * 16 + (k % 2) * 8:
                                    kb * 16 + (k % 2) * 8 + 8],
                            rhs=wh_sb[:, k, b * 1024 + c0:
                                      b * 1024 + c0 + cw],
                            start=False, stop=(k == KT - 1),
                            tile_position=(0, 32 * b),
                            skip_group_check=True,
                        )
            gp_fi, gp_g, gp_o = (t for t, _, _ in banks_cur)
            # next step's pre-gate rounds fill the PE during this step's
            # elementwise tail (independent of h)
            banks_next = (emit_sel(tau + 1)
                          if pipelined and tau + 1 < nc_T else None)
            # phase-1 work (or a dummy warm-keeper) as PE filler during
            # the elementwise tail: keeps the HAM clock-gate at 8/8
            emit_p1_step(tau)

            # --- elementwise phase, full width (all 4 bands at once;
            # non-band partitions compute garbage that stays confined) ---
            tg = ewpool.tile([128, 768], f32, tag="tg")
            so = ewpool.tile([128, 256], f32, tag="so")
            tc_t = ewpool.tile([128, 256], f32, tag="tc")
            a_t = ewpool.tile([128, 256], f32, tag="a")
            b_t = ewpool.tile([128, 256], f32, tag="b")
            # cols: [f | i | g~ | o]; f,i pre-scaled by 1/2 in W
            act.activation(tg[:, 0:512], gp_fi, Tanh)
            # A = (tf + 1) * S_prev      (S = 2*c)
            dve.scalar_tensor_tensor(
                a_t, tg[:, 0:256], 1.0, S_sb, op0=ADD, op1=MUL)
            act.activation(tg[:, 512:768], gp_g, Tanh)
            act.activation(so, gp_o, Sigm)
            # post-fi chain in 128-col chunks so h's first half (and the
            # kk=0 transpose) complete as early as possible
            for ch in range(2):
                sl = slice(ch * 128, (ch + 1) * 128)
                # Bv = (ti + 1) * tg~
                dve.scalar_tensor_tensor(
                    b_t[:, sl], tg[:, 256 + ch * 128:384 + ch * 128], 1.0,
                    tg[:, 512 + ch * 128:640 + ch * 128], op0=ADD, op1=MUL)
                # S_new = A * 0.5 + Bv   (= 2*c_new)
                dve.scalar_tensor_tensor(
                    S_sb[:, sl], a_t[:, sl], 0.5, b_t[:, sl],
                    op0=MUL, op1=ADD)
                # tc = tanh(c_new) = tanh(S_new / 2)
                act.activation(tc_t[:, sl], S_sb[:, sl], Tanh, scale=0.5)
                # h = sigmoid(o) * tanh(c)
                dve.tensor_mul(ring_f[:, slot, sl], so[:, sl], tc_t[:, sl])

                # --- transposes (band-major pairs only: alternating
                # row-group singles crash the device) ---
                if notrans or norec:
                    continue
                if ch == 0:
                    # band 0 kk=0 right after h's first half: unblocks
                    # next step's k=0 round
                    tp = tpool.tile([128, 64], f32, tag="tp", name="tp")
                    pe.transpose(
                        tp[:, 0:8],
                        in_=ring_f[0:8, slot, 0:128],
                        identity=id8_sb[0:8, :],
                        tile_position=(0, 0),
                    )
                    dve.tensor_copy(hT[:, 0:8], tp[:, 0:8])
                else:
                    pe.transpose(
                        tp[:, 8:16],
                        in_=ring_f[0:8, slot, 128:256],
                        identity=id8_sb[0:8, :],
                        tile_position=(0, 0),
                    )
                    dve.tensor_copy(hT[:, 8:16], tp[:, 8:16])
                    for b in range(1, 4):
                        rows = slice(32 * b, 32 * b + 8)
                        for kk in range(2):
                            pe.transpose(
                                tp[:, b * 16 + kk * 8:b * 16 + kk * 8 + 8],
                                in_=ring_f[rows, slot,
                                           kk * 128:(kk + 1) * 128],
                                identity=id8_sb[rows, :],
                                tile_position=(32 * b, 0),
                            )
                        dve.tensor_copy(hT[:, b * 16:b * 16 + 16],
                                        tp[:, b * 16:b * 16 + 16])

            if not nob:
                dve.tensor_mul(ring_b[:, bslot, :], so, tc_t)
            if pipelined:
                banks_cur = banks_next

            # --- output DMA every RING steps (partial at sequence end) ---
            if slot == RING - 1 or tau == nc_T - 1:
                cnt = slot + 1
                t0 = tau - slot
                bt0 = nc_T - 1 - tau
                for b in range(4):
                    sdma.dma_start(
                        out=outd[t0:t0 + cnt, 0, b, :, :]
                        .rearrange("t s n -> s t n"),
                        in_=ring_f[32 * b:32 * b + 4, 0:cnt, :],
                    )
                    sdma.dma_start(
                        out=outd[bt0:bt0 + cnt, 1, b, :, :]
                        .rearrange("t s n -> s t n"),
                        in_=ring_b[32 * b + 4:32 * b + 8,
                                   RING - cnt:RING, :],
                    )


def _prep_inputs(x, Wf, bf, Wi, bi, Wc, bc, Wo, bo, nc_T=T):
    """Host-side input preparation -> list of per-core in_maps."""
    # gate order (f, i, o, c~); f/i/o columns scaled by 1/2 (sigmoid via
    # tanh); Wh rows halved to absorb h being carried as 2*h.
    W = np.stack([Wf, Wi, Wc, Wo], axis=1)          # (2048, 4, 1024)
    bv = np.stack([bf, bi, bc, bo], axis=0)         # (4, 1024)
    scale = np.array([0.5, 0.5, 1.0, 1.0], dtype=np.float32)
    W = W * scale[None, :, None]
    bv = bv * scale[:, None]
    # column remap: col' = g*1024 + gate*256 + n for H index g*256+n
    W4 = np.ascontiguousarray(
        W.reshape(2048, 4, 4, 256).transpose(0, 2, 1, 3).reshape(2048, G4))
    b4 = np.ascontiguousarray(
        bv.reshape(4, 4, 256).transpose(1, 0, 2).reshape(G4))
    import ml_dtypes
    Wh = np.ascontiguousarray(W4[:H].astype(ml_dtypes.bfloat16))
    Wx = W4[H:]
    wxb = np.ascontiguousarray(
        np.concatenate([Wx, b4[None, :]], axis=0)
        .astype(ml_dtypes.bfloat16))  # (1025, 4096)
    # selection matrix rows (dir, hi/lo, g, bs); sums hi+lo parts
    selm = np.zeros((64, 32), dtype=np.float32)
    for b_ in range(4):
        for sp in range(8):
            for part in range(2):
                selm[(sp // 4) * 32 + part * 16 + b_ * 4 + (sp % 4),
                     b_ * 8 + sp] = 1.0
    selm = selm.astype(ml_dtypes.bfloat16)
    id8 = np.ascontiguousarray(np.tile(np.eye(8, dtype=np.float32), (16, 1)))

    in_maps = []
    for c in range(NCORES):
        # rows blocked by 32-step t-tile, s-major within the tile:
        # r = (t // 32) * 128 + s * 32 + t % 32
        xc = np.ascontiguousarray(
            x[BS * c:BS * c + BS, :nc_T, :]
            .reshape(BS, nc_T // 32, 32, D).transpose(1, 0, 2, 3)
        ).reshape(nc_T * BS, D)
        xTc = np.concatenate(
            [xc.T, np.ones((1, BS * nc_T), dtype=np.float32)], axis=0)
        in_maps.append({
            "xT": np.ascontiguousarray(xTc.astype(ml_dtypes.bfloat16)),
            "wxb": wxb,
            "wh": Wh,
            "sel": selm,
            "id8": id8,
        })
    return in_maps


def _assemble(results, nc_T=T):
    """results: list of dicts with 'out' (T, 2, 4, BS, 256) = 2*h."""
    full = np.empty((B, nc_T, 2 * H), dtype=np.float32)
    for c in range(NCORES):
        o = results[c]["out"]                       # (T, 2, 4, BS, 256)
        o2 = o.transpose(3, 0, 1, 2, 4).reshape(BS, nc_T, 2 * H)
        full[BS * c:BS * c + BS] = o2
    return full


def kernel(**inputs):
    global _BUILT
    from concourse.bass_utils import run_bass_kernel_spmd

    x = np.asarray(inputs["x"], dtype=np.float32)
    args = [np.asarray(inputs[k], dtype=np.float32)
            for k in ("Wf", "bf", "Wi", "bi", "Wc", "bc", "Wo", "bo")]
    in_maps = _prep_inputs(x, args[0], args[1], args[2], args[3],
                           args[4], args[5], args[6], args[7])
    if _BUILT is None:
        _BUILT = _build()
    res = run_bass_kernel_spmd(_BUILT, in_maps, core_ids=list(range(NCORES)))
    return _assemble(res.results)



# revision 41
# speedup vs baseline: 1.0537x; 1.0008x over previous
"""BiLSTM layer kernel for 8 Trainium2 NeuronCores.

Problem: B=32, T=256, D=1024, H=1024 bidirectional LSTM.
Sharding: data-parallel over batch (4 rows/core) x 2 directions = 8
sequences per core; weights replicated; time recurrence local per core.

Per-core algorithm:
  Phase 1: pre-gates PRE[s,t,:] = x[s,t,:] @ Wx + b (bf16, dense m=128
           matmuls) stored to a DRAM scratch tensor as bf16 hi+lo. The
           first 32 fwd/bwd steps' worth runs upfront; the rest is
           emitted one (m,n)-unit per recurrence step as PE filler that
           keeps the HAM clock-gate warm through the elementwise tails.
  Phase 2: 256 recurrent steps, fwd and bwd interleaved (8 sequences =
           4 batch x 2 directions share each Wh stream). Per step the
           gates accumulate into three psum tiles ([f,i], [g], [o]),
           round order fi -> g -> o, so tanh(f,i)+A overlap the g
           rounds and the g-chain (Bv, S, tc) overlaps the o rounds.
           h^T is the stationary operand (8 cols x 4 tile_position
           column strips, concurrent streams); the elementwise cell
           update runs full-width (all 4 bands at once, garbage lanes
           confined); h returns to stationary layout via PE transposes
           in band-major pairs (kk-major singles crash the device),
           band 0 first so next step's k=0 round starts early. The
           next step's pre-gate psum preload (sel rounds) is emitted
           before this step's transposes to avoid head-of-line
           blocking in the PE queue.

f and i gates go through tanh (sigmoid(x) = (tanh(x/2)+1)/2, the x/2
baked into the host-side weights, the (t+1)/2 folded into the cell
update with the state kept as S = 2*c); the o gate uses the ACT
sigmoid directly and h is carried plain.

Gate column order is (f, i, o, c~), grouped by h-quarter band:
  col = g*1024 + gate*256 + n   for h index  g*256 + n.
"""

import numpy as np

B, T, D, H = 32, 256, 1024, 1024
NCORES = 8
BS = B // NCORES           # batch rows per core
KT = D // 128              # k-tiles over the hidden/contraction dim
G4 = 4 * H                 # gate columns
RING = 8                   # output ring depth (steps per output DMA)

_BUILT = None


def _build(nc_T=T):
    import concourse.bass as bass
    import concourse.bacc as bacc
    import concourse.tile as tile
    from concourse import mybir

    f32 = mybir.dt.float32
    f32r = mybir.dt.float32r
    bf16 = mybir.dt.bfloat16
    Tanh = mybir.ActivationFunctionType.Tanh
    Sigm = mybir.ActivationFunctionType.Sigmoid
    MUL = mybir.AluOpType.mult
    ADD = mybir.AluOpType.add

    nrows = BS * nc_T      # rows of the phase-1 matmul

    nc = bacc.Bacc("TRN2", target_bir_lowering=False)

    xT = nc.dram_tensor("xT", [D + 1, nrows], bf16, kind="ExternalInput")
    wxb = nc.dram_tensor("wxb", [D + 1, G4], bf16, kind="ExternalInput")
    wh = nc.dram_tensor("wh", [D, G4], bf16, kind="ExternalInput")
    sel = nc.dram_tensor("sel", [64, 32], bf16, kind="ExternalInput")
    id8 = nc.dram_tensor("id8", [128, 8], f32, kind="ExternalInput")
    outd = nc.dram_tensor("out", [nc_T, 2, 4, BS, 256], f32,
                          kind="ExternalOutput")
    import os
    norec = bool(os.environ.get("K_NOREC"))
    nob = bool(os.environ.get("K_NOB"))
    notrans = bool(os.environ.get("K_NOTRANS"))
    pred = nc.dram_tensor("pre", [nc_T, 2, 4 * BS, 1024], bf16,
                          kind=("ExternalOutput"
                                if os.environ.get("K_P1DBG") else "Internal"))

    with tile.TileContext(nc) as tc:
        _emit(nc, tc, bass, mybir, nc_T,
              xT, wxb, wh, sel, id8, outd, pred,
              f32, f32r, bf16, Tanh, Sigm, MUL, ADD, norec, notrans,
              nob)
    nc.finalize()
    return nc


def _emit(nc, tc, bass, mybir, nc_T, xT, wxb, wh, sel, id8, outd, pred,
          f32, f32r, bf16, Tanh, Sigm, MUL, ADD,
          norec=False, notrans=False, nob=False):
    from contextlib import ExitStack

    nrows = BS * nc_T
    act = nc.scalar
    dve = nc.vector
    pool = nc.gpsimd
    pe = nc.tensor
    sdma = nc.sync

    with ExitStack() as ctx:
        # ------------- constants / big weights -------------
        singles = ctx.enter_context(tc.tile_pool(name="singles", bufs=1))
        sel_sb = singles.tile([64, 32], bf16)
        id8_sb = singles.tile([128, 8], f32)
        sdma.dma_start(out=sel_sb, in_=sel[:, :])
        sdma.dma_start(out=id8_sb, in_=id8[:, :])

        # ------------- persistent weights / inputs -------------
        wpool = ctx.enter_context(tc.tile_pool(name="wpool", bufs=1))
        wh_sb = wpool.tile([128, KT, G4], bf16, tag="wh")
        sdma.dma_start(
            out=wh_sb, in_=wh[:, :].rearrange("(k p) c -> p k c", p=128))
        wxb_sb = wpool.tile([128, KT, G4], bf16, tag="wx")
        sdma.dma_start(
            out=wxb_sb,
            in_=wxb[0:D, :].rearrange("(k p) c -> p k c", p=128))
        wxbrow = wpool.tile([1, G4], bf16, tag="wxr")
        sdma.dma_start(out=wxbrow, in_=wxb[D:D + 1, :])
        xT_sb = wpool.tile([128, KT, nrows], bf16, tag="xT")
        sdma.dma_start(
            out=xT_sb,
            in_=xT[0:D, :].rearrange("(k p) c -> p k c", p=128))
        ones_sb = wpool.tile([1, nrows], bf16, tag="ones")
        sdma.dma_start(out=ones_sb, in_=xT[D:D + 1, :])

        # ---------------- phase 1: pre-gates (x @ Wx + b) ----------------
        # t-major rows: row r = t * BS + s covers t-tile m = 32 t-steps.
        # m-tile order pairs (0,7),(1,6),... so the first pair covers the
        # first 32 fwd and bwd steps; one (m,n) unit is emitted per
        # recurrence step after the first 16 upfront units, keeping the
        # PE warm through the elementwise tails.
        p1out = ctx.enter_context(tc.tile_pool(name="p1out", bufs=3))
        p1ps = ctx.enter_context(tc.tile_pool(name="p1ps", bufs=1,
                                              space="PSUM"))

        n_mtiles = (nrows + 127) // 128
        morder = []
        for j in range((n_mtiles + 1) // 2):
            morder.append(j)
            if n_mtiles - 1 - j != j:
                morder.append(n_mtiles - 1 - j)

        p1_state = {}

        def emit_p1_part(m, n, part):
            """part 0: first 5 k-matmuls; part 1: rest + copies + DMA."""
            if part == 0:
                p1_state["pt"] = p1ps.tile([128, 512], f32, name="pt")
            pt = p1_state["pt"]
            krange = range(0, 5) if part == 0 else range(5, KT)
            for k in krange:
                pe.matmul(
                    pt,
                    lhsT=xT_sb[:, k, m * 128:(m + 1) * 128],
                    rhs=wxb_sb[:, k, n * 512:(n + 1) * 512],
                    start=(k == 0), stop=False,
                )
            if part == 0:
                return
            pe.matmul(
                pt,
                lhsT=ones_sb[:, m * 128:(m + 1) * 128],
                rhs=wxbrow[:, n * 512:(n + 1) * 512],
                start=False, stop=True,
            )
            ot = p1out.tile([128, 512], bf16, name="ot")
            ol = p1out.tile([128, 512], bf16, name="ol")
            dve.tensor_copy(ot, pt)
            dve.tensor_sub(ol, pt, ot)
            # rows r = m*128 + p -> (s = p // 32, t = 32*m + p % 32)
            g = n // 2
            half = n % 2
            t0 = 32 * m
            for s in range(BS):
                sdma.dma_start(
                    out=pred[t0:t0 + 32, 0, 4 * g + s,
                             half * 512:(half + 1) * 512],
                    in_=ot[s * 32:(s + 1) * 32, :],
                )
                sdma.dma_start(
                    out=pred[t0:t0 + 32, 1, 4 * g + s,
                             half * 512:(half + 1) * 512],
                    in_=ol[s * 32:(s + 1) * 32, :],
                )

        def emit_p1_dummy():
            """PE warm-keeper for steps past the phase-1 work: a few
            full-array matmuls into the (dead) p1 psum tile."""
            pt = p1ps.tile([128, 512], f32, name="pt")
            for k in range(4):
                pe.matmul(
                    pt,
                    lhsT=xT_sb[:, k, 0:128],
                    rhs=wh_sb[:, k, 0:512],
                    start=(k == 0), stop=(k == 3),
                )

        p1_units = [(m, n) for m in morder for n in range(8)]
        for m, n in p1_units[:16]:
            emit_p1_part(m, n, 0)
            emit_p1_part(m, n, 1)
        # units 16..47 one per step (steps 0..31), units 48..63 as
        # half-units (steps 32..63) -- keeps >=32 steps of slack between
        # a pred write and its first reader
        p1_sched = [(m, n, None) for m, n in p1_units[16:48]]
        p1_sched += [(m, n, p) for m, n in p1_units[48:] for p in (0, 1)]

        def emit_p1_step(tau):
            if tau < len(p1_sched):
                m, n, p = p1_sched[tau]
                if p is None:
                    emit_p1_part(m, n, 0)
                    emit_p1_part(m, n, 1)
                else:
                    emit_p1_part(m, n, p)

        # ---------------- phase 2: recurrence ----------------

        state = ctx.enter_context(tc.tile_pool(name="state", bufs=1))
        prepool = ctx.enter_context(tc.tile_pool(name="pre", bufs=2))
        ewpool = ctx.enter_context(tc.tile_pool(name="ew", bufs=2))
        ringpool = ctx.enter_context(tc.tile_pool(name="ring", bufs=1))
        gpool = ctx.enter_context(tc.tile_pool(name="gp", bufs=2,
                                               space="PSUM"))
        tpool = ctx.enter_context(tc.tile_pool(name="tp", bufs=1,
                                               space="PSUM"))

        # h^T stationary: one tile, band b k-subtile kk at cols b*16+kk*8
        hT = state.tile([128, 64], bf16, tag="hT", name="hT")
        # cell state as S = 2*c, per band rows, [8, 256] in band rows
        S_sb = state.tile([128, 256], f32, tag="S")
        dve.memset(S_sb, 0.0)

        ring_f = ringpool.tile([128, RING, 256], f32, tag="rf")
        ring_b = ringpool.tile([128, RING, 256], f32, tag="rb")

        def emit_sel(tau):
            """DMA pre-gates for step tau and preload the psum banks."""
            pfb = prepool.tile([64, 1024], bf16, tag="pfb")
            sdma.dma_start(out=pfb[0:32, :], in_=pred[tau, :, :, :])
            sdma.dma_start(out=pfb[32:64, :],
                           in_=pred[nc_T - 1 - tau, :, :, :])
            gp_fi = gpool.tile([128, 512], f32, tag="gfi", name="gfi")
            gp_g = gpool.tile([128, 256], f32, tag="gg", name="gg")
            gp_o = gpool.tile([128, 256], f32, tag="go", name="go")
            banks = ((gp_fi, 0, 512), (gp_g, 512, 256), (gp_o, 768, 256))
            for gt, c0, cw in banks:
                for b in range(4):
                    pe.matmul(
                        gt[32 * b:32 * b + 8, :],
                        lhsT=sel_sb[:, b * 8:(b + 1) * 8],
                        rhs=pfb[:, c0:c0 + cw],
                        start=True, stop=(tau == 0),
                        tile_position=(0, 32 * b),
                        skip_group_check=True,
                    )
            return banks

        # k-rounds in natural order: band-major transposes produce the
        # hT pieces in exactly this consumption order
        korder = list(range(KT))
        import os as _os
        pipelined = not bool(_os.environ.get("K_NOPIPE"))
        banks_cur = emit_sel(0) if pipelined else None

        for tau in range(nc_T):
            slot = tau % RING
            bslot = RING - 1 - slot
            if not pipelined:
                banks_cur = emit_sel(tau)

            # --- recurrent k-rounds, grouped fi -> g -> o ---
            for gt, c0, cw in banks_cur:
                for k in (korder if (tau > 0 and not norec) else []):
                    kb = k // 2  # band whose h feeds this k-tile
                    for b in range(4):
                        pe.matmul(
                            gt[32 * b:32 * b + 8, :],
                            lhsT=hT[:, kb * 16 + (k % 2) * 8:
                                    kb * 16 + (k % 2) * 8 + 8],
                            rhs=wh_sb[:, k, b * 1024 + c0:
                                      b * 1024 + c0 + cw],
                            start=False, stop=(k == KT - 1),
                            tile_position=(0, 32 * b),
                            skip_group_check=True,
                        )
            gp_fi, gp_g, gp_o = (t for t, _, _ in banks_cur)
            # next step's pre-gate rounds fill the PE during this step's
            # elementwise tail (independent of h)
            banks_next = (emit_sel(tau + 1)
                          if pipelined and tau + 1 < nc_T else None)
            # phase-1 work (or a dummy warm-keeper) as PE filler during
            # the elementwise tail: keeps the HAM clock-gate at 8/8
            emit_p1_step(tau)

            # --- elementwise phase, full width (all 4 bands at once;
            # non-band partitions compute garbage that stays confined) ---
            tg = ewpool.tile([128, 768], f32, tag="tg")
            so = ewpool.tile([128, 256], f32, tag="so")
            tc_t = ewpool.tile([128, 256], f32, tag="tc")
            a_t = ewpool.tile([128, 256], f32, tag="a")
            b_t = ewpool.tile([128, 256], f32, tag="b")
            # cols: [f | i | g~ | o]; f,i pre-scaled by 1/2 in W
            act.activation(tg[:, 0:512], gp_fi, Tanh)
            # A = (tf + 1) * S_prev      (S = 2*c)
            dve.scalar_tensor_tensor(
                a_t, tg[:, 0:256], 1.0, S_sb, op0=ADD, op1=MUL)
            act.activation(tg[:, 512:768], gp_g, Tanh)
            act.activation(so, gp_o, Sigm)
            # post-fi chain in 128-col chunks so h's first half (and the
            # kk=0 transpose) complete as early as possible
            for ch in range(2):
                sl = slice(ch * 128, (ch + 1) * 128)
                # Bv = (ti + 1) * tg~
                dve.scalar_tensor_tensor(
                    b_t[:, sl], tg[:, 256 + ch * 128:384 + ch * 128], 1.0,
                    tg[:, 512 + ch * 128:640 + ch * 128], op0=ADD, op1=MUL)
                # S_new = A * 0.5 + Bv   (= 2*c_new)
                dve.scalar_tensor_tensor(
                    S_sb[:, sl], a_t[:, sl], 0.5, b_t[:, sl],
                    op0=MUL, op1=ADD)
                # tc = tanh(c_new) = tanh(S_new / 2)
                act.activation(tc_t[:, sl], S_sb[:, sl], Tanh, scale=0.5)
                # h = sigmoid(o) * tanh(c)
                dve.tensor_mul(ring_f[:, slot, sl], so[:, sl], tc_t[:, sl])

                # --- transposes (band-major pairs only: alternating
                # row-group singles crash the device) ---
                if notrans or norec:
                    continue
                if ch == 0:
                    # band 0 kk=0 right after h's first half: unblocks
                    # next step's k=0 round
                    tp = tpool.tile([128, 64], f32, tag="tp", name="tp")
                    pe.transpose(
                        tp[:, 0:8],
                        in_=ring_f[0:8, slot, 0:128],
                        identity=id8_sb[0:8, :],
                        tile_position=(0, 0),
                    )
                    dve.tensor_copy(hT[:, 0:8], tp[:, 0:8])
                else:
                    pe.transpose(
                        tp[:, 8:16],
                        in_=ring_f[0:8, slot, 128:256],
                        identity=id8_sb[0:8, :],
                        tile_position=(0, 0),
                    )
                    dve.tensor_copy(hT[:, 8:16], tp[:, 8:16])
                    for b in range(1, 4):
                        rows = slice(32 * b, 32 * b + 8)
                        for kk in range(2):
                            pe.transpose(
                                tp[:, b * 16 + kk * 8:b * 16 + kk * 8 + 8],
                                in_=ring_f[rows, slot,
                                           kk * 128:(kk + 1) * 128],
                                identity=id8_sb[rows, :],
                                tile_position=(32 * b, 0),
                            )
                        dve.tensor_copy(hT[:, b * 16:b * 16 + 16],
                                        tp[:, b * 16:b * 16 + 16])

            if not nob:
                dve.tensor_mul(ring_b[:, bslot, :], so, tc_t)
            if pipelined:
                banks_cur = banks_next

            # --- output DMA every RING steps (partial at sequence end) ---
            if slot == RING - 1 or tau == nc_T - 1:
                cnt = slot + 1
                t0 = tau - slot
                bt0 = nc_T - 1 - tau
                for b in range(4):
                    sdma.dma_start(
                        out=outd[t0:t0 + cnt, 0, b, :, :]
                        .rearrange("t s n -> s t n"),
                        in_=ring_f[32 * b:32 * b + 4, 0:cnt, :],
                    )
                    sdma.dma_start(
                        out=outd[bt0:bt0 + cnt, 1, b, :, :]
                        .rearrange("t s n -> s t n"),
                        in_=ring_b[32 * b + 4:32 * b + 8,
                                   RING - cnt:RING, :],
                    )


def _prep_inputs(x, Wf, bf, Wi, bi, Wc, bc, Wo, bo, nc_T=T):
    """Host-side input preparation -> list of per-core in_maps."""
    # gate order (f, i, o, c~); f/i/o columns scaled by 1/2 (sigmoid via
    # tanh); Wh rows halved to absorb h being carried as 2*h.
    W = np.stack([Wf, Wi, Wc, Wo], axis=1)          # (2048, 4, 1024)
    bv = np.stack([bf, bi, bc, bo], axis=0)         # (4, 1024)
    scale = np.array([0.5, 0.5, 1.0, 1.0], dtype=np.float32)
    W = W * scale[None, :, None]
    bv = bv * scale[:, None]
    # column remap: col' = g*1024 + gate*256 + n for H index g*256+n
    W4 = np.ascontiguousarray(
        W.reshape(2048, 4, 4, 256).transpose(0, 2, 1, 3).reshape(2048, G4))
    b4 = np.ascontiguousarray(
        bv.reshape(4, 4, 256).transpose(1, 0, 2).reshape(G4))
    import ml_dtypes
    Wh = np.ascontiguousarray(W4[:H].astype(ml_dtypes.bfloat16))
    Wx = W4[H:]
    wxb = np.ascontiguousarray(
        np.concatenate([Wx, b4[None, :]], axis=0)
        .astype(ml_dtypes.bfloat16))  # (1025, 4096)
    # selection matrix rows (dir, hi/lo, g, bs); sums hi+lo parts
    selm = np.zeros((64, 32), dtype=np.float32)
    for b_ in range(4):
        for sp in range(8):
            for part in range(2):
                selm[(sp // 4) * 32 + part * 16 + b_ * 4 + (sp % 4),
                     b_ * 8 + sp] = 1.0
    selm = selm.astype(ml_dtypes.bfloat16)
    id8 = np.ascontiguousarray(np.tile(np.eye(8, dtype=np.float32), (16, 1)))

    in_maps = []
    for c in range(NCORES):
        # rows blocked by 32-step t-tile, s-major within the tile:
        # r = (t // 32) * 128 + s * 32 + t % 32
        xc = np.ascontiguousarray(
            x[BS * c:BS * c + BS, :nc_T, :]
            .reshape(BS, nc_T // 32, 32, D).transpose(1, 0, 2, 3)
        ).reshape(nc_T * BS, D)
        xTc = np.concatenate(
            [xc.T, np.ones((1, BS * nc_T), dtype=np.float32)], axis=0)
        in_maps.append({
            "xT": np.ascontiguousarray(xTc.astype(ml_dtypes.bfloat16)),
            "wxb": wxb,
            "wh": Wh,
            "sel": selm,
            "id8": id8,
        })
    return in_maps


def _assemble(results, nc_T=T):
    """results: list of dicts with 'out' (T, 2, 4, BS, 256) = 2*h."""
    full = np.empty((B, nc_T, 2 * H), dtype=np.float32)
    for c in range(NCORES):
        o = results[c]["out"]                       # (T, 2, 4, BS, 256)
        o2 = o.transpose(3, 0, 1, 2, 4).reshape(BS, nc_T, 2 * H)
        full[BS * c:BS * c + BS] = o2
    return full


def kernel(**inputs):
    global _BUILT
    from concourse.bass_utils import run_bass_kernel_spmd

    x = np.asarray(inputs["x"], dtype=np.float32)
    args = [np.asarray(inputs[k], dtype=np.float32)
            for k in ("Wf", "bf", "Wi", "bi", "Wc", "bc", "Wo", "bo")]
    in_maps = _prep_inputs(x, args[0], args[1], args[2], args[3],
                           args[4], args[5], args[6], args[7])
    if _BUILT is None:
        _BUILT = _build()
    res = run_bass_kernel_spmd(_BUILT, in_maps, core_ids=list(range(NCORES)))
    return _assemble(res.results)

